# revision 32
# baseline (speedup 1.0000x reference)
"""Trainium2 Bass kernel for nn_CustomNetwork (4-layer 4096x4096 MLP with
train-mode BatchNorm1d + ReLU per layer, batch-axis softmax at the end).

Strategy: data-parallel over the batch dim across 8 NeuronCores (512 rows
per core). Activations live in SBUF transposed (channels on partitions,
batch on the free dim) so BatchNorm stats and the batch-axis softmax are
native free-axis reductions. Matmuls run in fp16 (half the weight DMA of
fp32, and the GPIO power throttle pins the PE near 1.95 GHz either way, so
fp16's precision comes free vs bf16). BatchNorm statistics and softmax
sums stay in fp32. Weights are host-retiled to [L, KT, NSUP, P, 512] so
every weight-tile DMA is one 128 KiB contiguous block. PSUM is managed as
eight independent single-bank tiles so accumulation-group dependencies
stay per-bank.

The body is PE-bound and gapless (one MM every 512 PE cycles); the
schedule is built to keep the post-last-matmul tail minimal:
  - The device stores UNNORMALIZED exp values plus per-core local softmax
    sums; the host divides by the (host-reduced) global sums during
    unshard. This removes both softmax-sum collectives from the device
    critical path -- only BN-stat allreduces remain.
  - Mid layers use three stat chunks [0,24)/[24,28)/[28,32) whose
    allreduce round-trips hide under the next layer's matmuls.
  - The last layer chunks stats as [0,16)/[16,20)/[20,24) (each tile is
    exp'ed and stored as soon as its chunk lands), then [24,28) right
    after its supertile (read straight from held PSUM), and runs the
    final supertile tile-major with stats gathered as {28,29} (hidden
    under tile 30/31's matmuls) and {30,31}. The late stat groups use
    AllGather + local DVE tree-add, ~2x faster end-to-end than the CC
    AllReduce path, so only the tiny {30,31} gather plus two exps
    trails the last matmul.
  - Collective staging DMAs and the hidden early stores ride the
    Activation DGE queue; the critical tail staging/stores ride the SP
    queue, which is idle once the last weight block is in.

Note: the Linear bias `b` is mathematically canceled by BatchNorm's mean
subtraction, so it is never loaded.
"""

import numpy as np

import concourse.bacc as bacc
import concourse.mybir as mybir
import concourse.tile as tile
from concourse import bass_utils

P = 128  # SBUF partitions
D = 4096  # feature width
KT = D // P  # 32 k/n tiles
BM = 512  # per-core batch (4096 / 8 cores)
NSUP = 8  # n supertiles of 512 output channels
L = 4  # layers
N_CORES = 8
BN_EPS = 1e-5
# BN-stat allreduce chunks (mid layers).
CHUNKS = [(0, 24), (24, 28), (28, 32)]
# last layer: early chunks finish tiles [0,24) well before the final
# matmuls so their exp+store traffic all hides under the matmul phase
CHUNKS_LAST = [(0, 16), (16, 20), (20, 24)]

F32 = mybir.dt.float32
F16 = mybir.dt.float16

_cached_nc = None


def _bn_scale_shift(nc, small, red, gam_ap, bet_ap, n, tag):
    """From allreduced [P, 2, n] (sum of means, sum of E[h^2]) compute
    scale = gamma/sqrt(var+eps), shift = beta - mean*scale."""
    var_g = small.tile([P, n], F32, name=f"var_{tag}")
    scale = small.tile([P, n], F32, name=f"scale_{tag}")
    shift = small.tile([P, n], F32, name=f"shift_{tag}")
    # packs are pre-scaled by 1/N_CORES on each core, so the allreduced
    # values are already the global mean (red[:,0,:]) / E[h^2] (red[:,1,:])
    # var = E[h^2] - mean^2
    nc.vector.tensor_tensor(scale[:], red[:, 0, :], red[:, 0, :], op=mybir.AluOpType.mult)
    nc.vector.tensor_sub(var_g[:], red[:, 1, :], scale[:])
    nc.vector.tensor_scalar_add(var_g[:], var_g[:], BN_EPS)
    nc.scalar.activation(
        scale[:], var_g[:], mybir.ActivationFunctionType.Sqrt, bias=0.0, scale=1.0
    )
    nc.vector.reciprocal(scale[:], scale[:])
    nc.vector.tensor_mul(scale[:], scale[:], gam_ap)
    nc.vector.tensor_tensor(shift[:], red[:, 0, :], scale[:], op=mybir.AluOpType.mult)
    nc.vector.tensor_sub(shift[:], bet_ap, shift[:])
    return scale, shift


def _pack_stats(nc, small, meanvar, t0, t1, tag):
    """pack[:,0,:] = mean/8; pack[:,1,:] = (var + mean^2)/8 -- pre-scaled
    so the 8-core allreduce sum directly yields global mean / E[h^2]."""
    n = t1 - t0
    pack = small.tile([P, 2, n], F32, name=f"pack_{tag}")
    nc.vector.tensor_scalar_mul(pack[:, 0, :], meanvar[:, t0:t1, 0], 1.0 / N_CORES)
    nc.vector.tensor_tensor(
        pack[:, 1, :], meanvar[:, t0:t1, 0], meanvar[:, t0:t1, 0],
        op=mybir.AluOpType.mult,
    )
    nc.vector.tensor_tensor(
        pack[:, 1, :], pack[:, 1, :], meanvar[:, t0:t1, 1], op=mybir.AluOpType.add
    )
    nc.vector.tensor_scalar_mul(pack[:, 1, :], pack[:, 1, :], 1.0 / N_CORES)
    return pack


def build():
    global _cached_nc
    if _cached_nc is not None:
        return _cached_nc
    nc = bacc.Bacc("TRN2", target_bir_lowering=False, debug=False, num_devices=N_CORES)

    xt = nc.dram_tensor("xt", [D, BM], F16, kind="ExternalInput")
    # host-retiled weights: [l, k, ns] tile is a contiguous [P, 512] block
    Wt = nc.dram_tensor("W", [L, KT, NSUP, P, 512], F16, kind="ExternalInput")
    # gammaH/betaH are host-transposed to [L, P, KT] so the DMA runs with
    # contiguous lines
    gamma = nc.dram_tensor("gammaH", [L, P, KT], F32, kind="ExternalInput")
    beta = nc.dram_tensor("betaH", [L, P, KT], F32, kind="ExternalInput")
    # unnormalized exp(relu(bn(h))) of the last layer, [channels, batch]
    outt = nc.dram_tensor("outt", [D, BM], F16, kind="ExternalOutput")
    # per-core local softmax sums, [P, KT]: channel t*128+p at [p, t]
    sums = nc.dram_tensor("sums", [P, KT], F32, kind="ExternalOutput")

    rg = [list(range(N_CORES))]

    def allreduce(pool_dram, src_ap, dst_tile, tag, eng=None):
        # staging DMAs ride the Activation queue by default so collective
        # inputs/results never wait behind bulk weight traffic on SP.
        eng = eng or nc.scalar
        ar_in = pool_dram.tile(list(src_ap.shape), F32, name=f"arin_{tag}")
        ar_out = pool_dram.tile(list(src_ap.shape), F32, name=f"arout_{tag}")
        eng.dma_start(ar_in[:], src_ap)
        nc.gpsimd.collective_compute(
            "AllReduce",
            mybir.AluOpType.add,
            replica_groups=rg,
            ins=[ar_in.opt()],
            outs=[ar_out.opt()],
        )
        eng.dma_start(dst_tile[:], ar_out[:])

    with tile.TileContext(nc) as tc:
        with (
            tc.tile_pool(name="hbuf", bufs=1) as hpool,
            tc.tile_pool(name="wpool", bufs=64) as wpool,
            tc.tile_pool(name="w7", bufs=1) as w7pool,
            tc.tile_pool(name="psum", bufs=8, space="PSUM") as psum,
            tc.tile_pool(name="small", bufs=2) as small,
            tc.tile_pool(name="gb", bufs=1) as gbpool,
            tc.tile_pool(name="dram", bufs=1, space="DRAM") as dram,
        ):
            h = [
                hpool.tile([P, KT, BM], F16, name="h_a"),
                hpool.tile([P, KT, BM], F16, name="h_b"),
            ]

            # x^T -> h[0] on the Activation DGE queue while the SP queue
            # streams layer-0 weights, so each k's (x, w) pair lands
            # together and the PE can start streaming early.
            w_pre = []
            for k in range(KT):
                nc.scalar.dma_start(h[0][:, k, :], xt.ap()[k * P : (k + 1) * P, :])
                wt = wpool.tile([P, 512], F16, name="wt")
                nc.sync.dma_start(wt[:], Wt.ap()[0, k, 0])
                w_pre.append(wt)

            gam = gbpool.tile([P, L, KT], F32, name="gam")
            bet = gbpool.tile([P, L, KT], F32, name="bet")
            for l in range(L):
                nc.gpsimd.dma_start(gam[:, l, :], gamma.ap()[l])
                nc.gpsimd.dma_start(bet[:, l, :], beta.ap()[l])

            sumexp = small.tile([P, KT], F32, name="sumexp")
            # last layer's final supertile weights, preloaded for the
            # tile-major pass
            wt7 = w7pool.tile([P, KT, 512], F16, name="wt7")

            for l in range(L):
                last = l == L - 1
                src = h[l % 2]
                dst = h[(l + 1) % 2]

                stat6 = small.tile([P, KT, 6], F32, name=f"stat6_{l}")
                meanvar = small.tile([P, KT, 2], F32, name=f"meanvar_{l}")

                def gather_stats(t0, t1, tag, eng_a, eng_b, meanvar=meanvar):
                    """Cross-core stat reduction via AllGather + local DVE
                    tree-add: ~2x faster doorbell-to-result than the CC
                    AllReduce path, with far fewer straggler slices. Used
                    wherever the round-trip margin is tight."""
                    n = t1 - t0
                    pack = _pack_stats(nc, small, meanvar, t0, t1, tag)
                    ag_in = dram.tile([P, 2, n], F32, name=f"agin_{tag}")
                    ag_out = dram.tile(
                        [N_CORES, P, 2, n], F32, name=f"agout_{tag}"
                    )
                    eng_a.dma_start(ag_in[:], pack[:])
                    nc.gpsimd.collective_compute(
                        "AllGather",
                        mybir.AluOpType.bypass,
                        replica_groups=rg,
                        ins=[ag_in.opt()],
                        outs=[ag_out.opt()],
                    )
                    redG = small.tile(
                        [P, N_CORES, 2, n], F32, name=f"redg_{tag}"
                    )
                    for r in range(N_CORES):
                        eng = eng_a if r % 2 == 0 else eng_b
                        eng.dma_start(redG[:, r, :, :], ag_out[r])
                    red = small.tile([P, 2, n], F32, name=f"red_{tag}")
                    nc.vector.tensor_tensor(
                        redG[:, 0:4], redG[:, 0:4], redG[:, 4:8],
                        op=mybir.AluOpType.add,
                    )
                    nc.vector.tensor_tensor(
                        redG[:, 0:2], redG[:, 0:2], redG[:, 2:4],
                        op=mybir.AluOpType.add,
                    )
                    nc.vector.tensor_tensor(
                        red[:], redG[:, 0, :, :], redG[:, 1, :, :],
                        op=mybir.AluOpType.add,
                    )
                    return red

                # ---- matmul phase: out^T[n, m] = sum_k W[k, n] * h^T[k, m]
                ps_hold = {}
                n_seq = NSUP - 1 if last else NSUP
                for ns in range(n_seq):
                    ps = [psum.tile([P, BM], F32, name="ps") for _ in range(4)]
                    for k in range(KT):
                        if l == 0 and ns == 0:
                            wt = w_pre[k]
                        else:
                            wt = wpool.tile([P, 512], F16, name="wt")
                            nc.sync.dma_start(wt[:], Wt.ap()[l, k, ns])
                        for j in range(4):
                            nc.tensor.matmul(
                                ps[j][:],
                                wt[:, j * P : (j + 1) * P],
                                src[:, k, :],
                                start=(k == 0),
                                stop=(k == KT - 1),
                            )
                    if last and ns == 4:
                        # preload ns=7's weights for the tile-major pass
                        for k in range(KT):
                            nc.sync.dma_start(wt7[:, k, :], Wt.ap()[l, k, 7])
                    # On the last layer, tiles 24..27 skip the pre-BN copy:
                    # the exp-apply reads straight from PSUM (nothing needs
                    # those banks afterwards).
                    hold = last and ns == 6
                    for j in range(4):
                        t = ns * 4 + j
                        nc.vector.bn_stats(stat6[:, t, :], ps[j][:])
                        nc.vector.bn_aggr(meanvar[:, t, :], stat6[:, t, :])
                        if not hold:
                            nc.vector.tensor_copy(dst[:, t, :], ps[j][:])
                    if hold:
                        ps_hold[6] = ps
                if last:
                    # ns=7 tile-major: each tile's 32-step k-loop runs to
                    # completion so its BN stats start while the next tile's
                    # matmuls run; only tile 31's stats trail the last matmul.
                    ps = [psum.tile([P, BM], F32, name="ps") for _ in range(4)]
                    for j in range(4):
                        t = 28 + j
                        for k in range(KT):
                            nc.tensor.matmul(
                                ps[j][:],
                                wt7[:, k, j * P : (j + 1) * P],
                                src[:, k, :],
                                start=(k == 0),
                                stop=(k == KT - 1),
                            )
                        nc.vector.bn_stats(stat6[:, t, :], ps[j][:])
                        nc.vector.bn_aggr(meanvar[:, t, :], stat6[:, t, :])
                    ps_hold[7] = ps

                if not last:
                    # ---- BN: chunked cross-core mean / E[h^2] allreduce +
                    # fused scale/shift/relu apply
                    for ci, (t0, t1) in enumerate(CHUNKS):
                        n = t1 - t0
                        tag = f"{l}_{ci}"
                        if ci == len(CHUNKS) - 1:
                            # the last chunk's result is needed only ~29us
                            # after the layer's final matmul -- the fast
                            # gather path keeps that margin positive even
                            # under cross-rank skew
                            red = gather_stats(t0, t1, tag, nc.scalar, nc.sync)
                        else:
                            pack = _pack_stats(nc, small, meanvar, t0, t1, tag)
                            red = small.tile([P, 2, n], F32, name=f"red_{tag}")
                            allreduce(dram, pack[:], red, tag)
                        scale, shift = _bn_scale_shift(
                            nc, small, red, gam[:, l, t0:t1], bet[:, l, t0:t1], n, tag
                        )
                        for i in range(n):
                            t = t0 + i
                            nc.scalar.activation(
                                dst[:, t, :],
                                dst[:, t, :],
                                mybir.ActivationFunctionType.Relu,
                                bias=shift[:, i : i + 1],
                                scale=scale[:, i : i + 1],
                            )
                else:
                    # ---- last layer: BN + exp; store UNNORMALIZED exp and
                    # the local per-channel exp-sums (host does the softmax
                    # divide during unshard).
                    # exp(relu(z)) = max(exp(z), 1); the DVE max also
                    # accumulates the per-channel exp-sum.
                    def exp_store(t, src_ap, scale, shift, i, store_eng=nc.scalar):
                        nc.scalar.activation(
                            dst[:, t, :],
                            src_ap,
                            mybir.ActivationFunctionType.Exp,
                            bias=shift[:, i : i + 1],
                            scale=scale[:, i : i + 1],
                        )
                        nc.vector.tensor_scalar(
                            dst[:, t, :],
                            dst[:, t, :],
                            1.0,
                            0.0,
                            mybir.AluOpType.max,
                            mybir.AluOpType.add,
                            accum_out=sumexp[:, t : t + 1],
                        )
                        store_eng.dma_start(
                            outt.ap()[t * P : (t + 1) * P, :], dst[:, t, :]
                        )

                    # tiles [0,24): chunked stat allreduces, all landing
                    # during the matmul phase
                    for ci, (t0, t1) in enumerate(CHUNKS_LAST):
                        n = t1 - t0
                        tag = f"L_{ci}"
                        pack = _pack_stats(nc, small, meanvar, t0, t1, tag)
                        red = small.tile([P, 2, n], F32, name=f"red_{tag}")
                        allreduce(dram, pack[:], red, tag)
                        scale, shift = _bn_scale_shift(
                            nc, small, red, gam[:, l, t0:t1], bet[:, l, t0:t1], n, tag
                        )
                        for i in range(n):
                            exp_store(t0 + i, dst[:, t0 + i, :], scale, shift, i)

                    # tiles [24,28): stats ready one supertile before the
                    # end; gather lands pre-last-matmul, exp reads PSUM
                    redC = gather_stats(24, 28, "nsC", nc.scalar, nc.sync)
                    scaleC, shiftC = _bn_scale_shift(
                        nc, small, redC, gam[:, l, 24:28], bet[:, l, 24:28], 4, "nsC"
                    )
                    for i in range(4):
                        exp_store(24 + i, ps_hold[6][i][:], scaleC, shiftC, i)

                    # tiles {28,29}: their gather is triggered as soon as
                    # tile 29's k-loop retires (~17us before the last
                    # matmul), so the round-trip and both exps hide under
                    # tile 30/31's matmuls. Only {30,31} trail the last
                    # matmul: one small AllGather + two exps is the whole
                    # critical tail. SP is idle throughout ns=7 (wt7 was
                    # preloaded), so staging/stores ride it freely.
                    redY = gather_stats(28, 30, "nsY", nc.sync, nc.scalar)
                    scaleY, shiftY = _bn_scale_shift(
                        nc, small, redY, gam[:, l, 28:30], bet[:, l, 28:30], 2, "nsY"
                    )
                    for i in range(2):
                        exp_store(28 + i, ps_hold[7][i][:], scaleY, shiftY, i,
                                  store_eng=nc.sync)

                    redW = gather_stats(30, 32, "nsW", nc.sync, nc.scalar)
                    scaleW, shiftW = _bn_scale_shift(
                        nc, small, redW, gam[:, l, 30:32], bet[:, l, 30:32], 2, "nsW"
                    )
                    exp_store(30, ps_hold[7][2][:], scaleW, shiftW, 0,
                              store_eng=nc.sync)
                    # tile 31: sums store goes out between its accum and its
                    # (larger) exp store
                    nc.scalar.activation(
                        dst[:, 31, :],
                        ps_hold[7][3][:],
                        mybir.ActivationFunctionType.Exp,
                        bias=shiftW[:, 1:2],
                        scale=scaleW[:, 1:2],
                    )
                    nc.vector.tensor_scalar(
                        dst[:, 31, :],
                        dst[:, 31, :],
                        1.0,
                        0.0,
                        mybir.AluOpType.max,
                        mybir.AluOpType.add,
                        accum_out=sumexp[:, 31:32],
                    )
                    # local softmax sums out (host reduces across cores)
                    nc.sync.dma_start(sums.ap()[:, :], sumexp[:])
                    nc.sync.dma_start(outt.ap()[31 * P : 32 * P, :], dst[:, 31, :])

    nc.compile()
    _cached_nc = nc
    return nc


def make_in_maps(x, W, gamma, beta):
    """Host-side prep: shard x over the batch dim, transpose to [D, BM],
    convert the matmul operands to fp16 (weights also retiled so each
    [P, 512] tile is contiguous), transpose gamma/beta to [L, P, KT]."""
    x = np.asarray(x, dtype=np.float32)
    W = np.asarray(W, dtype=np.float32)
    gamma = np.asarray(gamma, dtype=np.float32)
    beta = np.asarray(beta, dtype=np.float32)
    # W[l, k*P+p, ns*512+c] -> Wtiled[l, k, ns, p, c]
    Wtiled = np.empty((L, KT, NSUP, P, 512), dtype=np.float16)
    Wtiled[...] = np.ascontiguousarray(W).reshape(L, KT, P, NSUP, 512).transpose(
        0, 1, 3, 2, 4
    )
    # [L, D] -> [L, P, KT]: channel (t*128 + p) lands at [l, p, t]
    gammaH = np.ascontiguousarray(gamma.reshape(L, KT, P).transpose(0, 2, 1))
    betaH = np.ascontiguousarray(beta.reshape(L, KT, P).transpose(0, 2, 1))
    in_maps = []
    for c in range(N_CORES):
        xt_c = np.ascontiguousarray(x[c * BM : (c + 1) * BM, :].T.astype(np.float16))
        in_maps.append(
            {"xt": xt_c, "W": Wtiled, "gammaH": gammaH, "betaH": betaH}
        )
    return in_maps


def kernel(x, W, b, gamma, beta):
    """Full (unsharded) inputs -> full [4096, 4096] softmax output."""
    del b  # canceled by BatchNorm mean subtraction
    nc = build()
    in_maps = make_in_maps(x, W, gamma, beta)
    r = bass_utils.run_bass_kernel_spmd(nc, in_maps, core_ids=list(range(N_CORES)))
    # global softmax denominator: sum the per-core local sums on host.
    # sums[p, t] is channel t*128+p -> flatten to [D] in channel order.
    total = np.zeros((P, KT), dtype=np.float32)
    for c in range(N_CORES):
        total += r.results[c]["sums"]
    denom = total.T.reshape(D)  # [KT, P] -> channel t*128+p
    inv = (1.0 / denom).astype(np.float32)
    out = np.empty((N_CORES * BM, D), dtype=np.float32)
    for c in range(N_CORES):
        out[c * BM : (c + 1) * BM, :] = (
            r.results[c]["outt"].T.astype(np.float32) * inv[None, :]
        )
    return out
